# revision 25
# baseline (speedup 1.0000x reference)
"""Trainium2 Bass kernel for nn_CustomSVRActivationLayer_39934605918594.

Reference semantics (B=4096 rows, F=8192 features, table = weights_matrix[0,:,0]):
    y_i    = table[true_label_i]
    sx_i   = sum_j x_ij ;  a_i = sx_i / B              (note: divides by B, not F)
    y_avg  = sum_i y_i / B
    Sx_i   = sum_j (x_ij - a_i)^2
    Sxy_i  = (y_i - y_avg) * sum_j (x_ij - a_i)
    beta_i = Sxy_i / Sx_i ; alpha_i = y_avg - beta_i * a_i
    upd_i  = beta_i * sx_i + alpha_i
    out    = broadcast(upd, [B, 1000])

Because F = 2B exactly:  sum_j (x_ij - a_i) = -sx_i  and  Sx_i = sum_j x_ij^2,
so with per-row feature mean m_i and population variance v_i over the F axis:
    upd_i = y_avg - 8190 * (y_i - y_avg) * m_i^2 / (v_i + m_i^2)
(8192 * 4095/4096 = 8190). Verified: rel-l2 ~5e-7 vs the jax reference.

Sharding: data-parallel over B. Each of the 8 cores gets 512 rows of
`inputs`, the full label vector (rotated so its own 512 labels come first),
and the full weights_matrix; it computes mean/var per row with bn_stats
(single streaming pass over its 16 MB shard), gathers y for all 4096 labels
from the replicated 1000-entry table with one gpsimd ap_gather (so the
global sum(y) is computed redundantly per core - no collective needed),
and writes its 512x1000 broadcast output block.

Queue layout (the two HW DGE rings are in-order FIFOs): the bulk x-stream
owns the SP ring; the ACT ring carries table/idx loads up front and the
output writes at the end. The epilogue runs per row-block so output
broadcasts/writes overlap later row-block streaming. sum(y) is replicated
to all partitions in one step via ones[128,128].T @ colsum into PSUM.
"""

import numpy as np

import concourse.bacc as bacc
import concourse.mybir as mybir
import concourse.tile as tile
from concourse import bass
from concourse.bass_utils import run_bass_kernel_spmd

B, F = 4096, 8192
NCLS = 1000
NCORES = 8
RPC = B // NCORES          # rows per core = 512
RB = RPC // 128            # 128-row blocks per core = 4
FCH = 2048                 # feature chunk per DMA
NCH = F // FCH             # chunks per row block = 4
SUB = 512                  # bn_stats subgroup size
F32 = mybir.dt.float32

_cache = {}


def _build():
    nc = bacc.Bacc("TRN2", target_bir_lowering=False, debug=False,
                   enable_asserts=False, num_devices=NCORES)

    x = nc.dram_tensor("x", [RPC, F], F32, kind="ExternalInput").ap()
    wm = nc.dram_tensor("wm", [NCLS, NCLS], F32, kind="ExternalInput").ap()
    lab = nc.dram_tensor("lab", [B], mybir.dt.int32, kind="ExternalInput")
    out = nc.dram_tensor("out", [RPC, NCLS], F32, kind="ExternalOutput").ap()

    lab16 = lab.ap().bitcast(mybir.dt.int16)  # [8192] int16, low half first (LE)

    with tile.TileContext(nc) as tc:
        with (
            tc.tile_pool(name="singles", bufs=1) as singles,
            tc.tile_pool(name="xp", bufs=8) as xp,
            tc.tile_pool(name="statp", bufs=4) as statp,
            tc.tile_pool(name="outp", bufs=4) as outp,
            tc.tile_pool(name="epip", bufs=8) as epip,
            tc.tile_pool(name="psum", bufs=1, space="PSUM") as psump,
        ):
            # ---- bulk x-stream: emit first so the SP queue leads with it ----
            # The last row-block tapers its chunks so the tail
            # (last-DMA -> bn_stats -> epilogue -> out) is shorter.
            def chunks_of(rb):
                if rb == RB - 1:
                    return [2048, 2048, 2048, 1536, 512]
                return [FCH] * NCH

            stream = [(rb, ci, sum(chunks_of(rb)[:ci]), w)
                      for rb in range(RB) for ci, w in enumerate(chunks_of(rb))]
            xtiles = {}

            def emit_x(item):
                rb, ci, off, w = item
                xt = xp.tile([128, w], F32, tag="xs" if w < FCH else "x")
                nc.sync.dma_start(
                    out=xt[:, :],
                    in_=x[rb * 128:(rb + 1) * 128, off:off + w],
                )
                xtiles[(rb, ci)] = (xt, off, w)

            for item in stream[:6]:
                emit_x(item)

            # ---- y pipeline ----
            # Load the 1000-entry table to one partition per 16-partition
            # gather group only (zeros elsewhere): unused partitions then
            # gather exact zeros, so the all-partition sum needs no /16.
            table_rep = singles.tile([128, NCLS], F32)
            nc.vector.memset(table_rep[:, :], 0.0)
            nc.scalar.dma_start(out=table_rep[0:128:16, :],
                                in_=wm[0:1, :].partition_broadcast(NCORES))

            ones_sq = singles.tile([128, 128], F32)
            nc.vector.memset(ones_sq[:, :], 1.0)
            ones_bc = singles.tile([128, NCLS], F32)
            nc.vector.memset(ones_bc[:, :], 1.0)

            # wrapped gather indices: idx[16g+r, c] = label[g*512 + c*16 + r]
            idxs = singles.tile([128, B // 128], mybir.dt.int16)
            for g in range(NCORES):
                idx_src = bass.AP(tensor=lab16.tensor, offset=g * 2 * RPC,
                                  ap=[[2, 16], [32, B // 128]])
                nc.scalar.dma_start(out=idxs[16 * g:16 * (g + 1), :], in_=idx_src)

            for item in stream[6:]:
                emit_x(item)

            # gather y for all 4096 labels (each 16-partition group: 512 of them)
            y_all = singles.tile([128, RPC], F32)
            nc.gpsimd.ap_gather(
                out_ap=y_all[:, :].unsqueeze(-1),
                in_ap=table_rep[:, :].unsqueeze(-1),
                idxs_ap=idxs[:, :],
                channels=128, num_elems=NCLS, d=1, num_idxs=RPC,
            )

            # global sum(y): free-reduce, then ones[128,128].T @ colsum puts the
            # partition-sum on every PSUM partition at once (no broadcast
            # round-trip). Only one partition per group gathered real values.
            colsum = singles.tile([128, 1], F32)
            nc.vector.reduce_sum(out=colsum[:, :], in_=y_all[:, :],
                                 axis=mybir.AxisListType.X)
            ps = psump.tile([128, 1], F32)
            nc.tensor.matmul(ps[:, :], ones_sq[:, :], colsum[:, :],
                             start=True, stop=True)
            y_avg_rep = singles.tile([128, 1], F32)
            nc.vector.tensor_scalar(y_avg_rep[:, :], ps[:, :],
                                    1.0 / B, None,
                                    mybir.AluOpType.mult)

            # own y per partition: labels were rotated host-side so this core's
            # rows are global slots 0..511 -> group 0. y_own[p, rb] = y_all[0, rb*128+p]
            y_own = singles.tile([128, RB], F32)
            for rb in range(RB):
                nc.scalar.dma_start(out=y_own[:, rb:rb + 1],
                                    in_=y_all[0:1, rb * 128:(rb + 1) * 128])

            # ---- per row-block: stats, epilogue, broadcast, write ----
            for rb in range(RB):
                stats = statp.tile([128, (F // SUB), 6], F32)
                si = 0
                for ci, w in enumerate(chunks_of(rb)):
                    xt, off, w = xtiles[(rb, ci)]
                    for s in range(w // SUB):
                        nc.vector.bn_stats(
                            out=stats[:, si, :],
                            in_=xt[:, s * SUB:(s + 1) * SUB],
                        )
                        si += 1
                assert si == F // SUB
                mv = epip.tile([128, 2], F32)
                nc.vector.bn_aggr(
                    out=mv[:, :],
                    in_=stats[:, :, :].rearrange("p n (a b) -> p (n a) b", b=3),
                )
                # upd = y_avg - 8190*(y - y_avg)*m^2/(v + m^2), all [128,1]
                mean, var = mv[:, 0:1], mv[:, 1:2]
                den = epip.tile([128, 1], F32)
                nc.vector.tensor_scalar(den[:, :], mean, mean, var,
                                        mybir.AluOpType.mult, mybir.AluOpType.add)
                rden = epip.tile([128, 1], F32)
                nc.vector.reciprocal(out=rden[:, :], in_=den[:, :])
                q = epip.tile([128, 1], F32)
                nc.vector.tensor_scalar(q[:, :], rden[:, :], mean, mean,
                                        mybir.AluOpType.mult,
                                        mybir.AluOpType.mult)
                t = epip.tile([128, 1], F32)
                nc.vector.tensor_scalar(t[:, :], y_own[:, rb:rb + 1],
                                        y_avg_rep[:, :], q[:, :],
                                        mybir.AluOpType.subtract,
                                        mybir.AluOpType.mult)
                upd = epip.tile([128, 1], F32)
                nc.vector.tensor_scalar(upd[:, :], t[:, :], -8190.0,
                                        y_avg_rep[:, :],
                                        mybir.AluOpType.mult, mybir.AluOpType.add)

                # split the last write so broadcast/descr/transfer pipeline;
                # its halves use both DGE rings (idle by then) and DVE (idle
                # after the epilogue) for the broadcast.
                halves = 2 if rb == RB - 1 else 1
                hw_ = NCLS // halves
                for h in range(halves):
                    ob = outp.tile([128, hw_], F32, tag="ob2" if halves == 2 else "ob")
                    if halves == 2:
                        nc.vector.tensor_scalar(ob[:, :], ones_bc[:, 0:hw_],
                                                upd[:, :], None,
                                                mybir.AluOpType.mult)
                        eng = nc.sync if h == 0 else nc.scalar
                    else:
                        nc.scalar.mul(ob[:, :], ones_bc[:, 0:hw_], upd[:, :])
                        # rb2's write rides the SP ring: in-order behind all x
                        # chunks, it cannot preempt rb3's final small chunks.
                        eng = nc.sync if rb == RB - 2 else nc.scalar
                    eng.dma_start(
                        out=out[rb * 128:(rb + 1) * 128, h * hw_:(h + 1) * hw_],
                        in_=ob[:, :],
                    )

    nc.compile()
    return nc


def get_nc():
    if "nc" not in _cache:
        _cache["nc"] = _build()
    return _cache["nc"]


def make_in_maps(inputs, weights_matrix, true_label):
    x = np.ascontiguousarray(np.asarray(inputs, dtype=np.float32))
    wm = np.ascontiguousarray(np.asarray(weights_matrix, dtype=np.float32)
                              .reshape(NCLS, NCLS))
    lab = np.asarray(true_label)
    assert lab.shape == (B,)
    lab32 = np.ascontiguousarray(lab.astype(np.int32, copy=False))
    in_maps = []
    for k in range(NCORES):
        in_maps.append({
            "x": x[k * RPC:(k + 1) * RPC],
            "wm": wm,
            "lab": np.ascontiguousarray(np.roll(lab32, -k * RPC)),
        })
    return in_maps


def kernel(inputs, weights_matrix, true_label, _trace=False):
    nc = get_nc()
    in_maps = make_in_maps(inputs, weights_matrix, true_label)
    res = run_bass_kernel_spmd(nc, in_maps, core_ids=list(range(NCORES)),
                               trace=_trace)
    outs = [res.results[k]["out"] for k in range(NCORES)]
    full = np.concatenate(outs, axis=0)
    if _trace:
        _cache["last_results"] = res
    return full
